# revision 8
# baseline (speedup 1.0000x reference)
"""Bass/Tile TRN2 kernel for nn_AttentionANEWraperChannelsFirstWithCache.

Tensor-parallel over heads across 8 NeuronCores (v2):
  - 28 q heads in 4 slots/core; core c owns kv head c//2 (replicated per pair).
  - Head-slot groups processed sequentially (slot 0,1,2,3), each over the full
    4096-row cache with the cache-update window tiles ordered last so
    attention starts before the k/v projections finish.
  - exp chunks of [128, 1024] (2 s-tiles) on the scalar engine, double
    buffered in PSUM; softmax denominator accumulated on DVE in bf16;
    per-slot biases applied on DVE (tensor_scalar_add).
  - AllGather per slot-pair {0,1} / slot {2} / slot {3}; the first two overlap
    later attention; o_proj accumulates in 2 rotating PSUM banks with DVE
    flushes into an SBUF accumulator, interleaved into slot-3 attention.
  - K cache pre-transposed on host ([d, s]); x/wq/wk/wv/v-cache/woT laid out
    host-side so every DMA is contiguous per partition.

Matmul operands bf16 (fp32 PSUM), softmax stats fp32/bf16 mix.
"""

import math
import numpy as np

H, KV, HD, LI = 28, 4, 128, 5
S_MAX, D, L = 4096, 3584, 512
NCORES = 8
SLOTS = 4
OSH = D // NCORES          # 448 o_proj output rows per core
NT = D // 128              # 28 contraction tiles over hidden dim
ST = S_MAX // 128          # 32 s-tiles over the cache
SCALE = 1.0 / math.sqrt(HD)


def _head_of(core, slot):
    off = 4 * (core % 2) + slot
    if off >= 7:
        return None                      # dummy slot (odd cores, slot 3)
    return (core // 2) * 7 + off


# o_proj entry order = gather-buffer order: group A (slots 0,1 on all cores),
# group B (slot 2 on all cores), group C (slot 3, even cores only).
ENTRIES = ([("A", c, h) for c in range(NCORES) for h in (0, 1)]
           + [("B", c, 2) for c in range(NCORES)]
           + [("C", c, 3) for c in range(0, NCORES, 2)])
assert len(ENTRIES) == H

_prog_cache = {}


def _build(cp):
    import concourse.bass as bass
    import concourse.mybir as mybir
    import concourse.tile as tile
    from concourse import bacc
    from contextlib import ExitStack

    f32 = mybir.dt.float32
    bf = mybir.dt.bfloat16
    AF = mybir.ActivationFunctionType
    nc = bacc.Bacc("TRN2", target_bir_lowering=False, debug=False,
                   num_devices=NCORES)

    x_d = nc.dram_tensor("x", [128, NT * L], bf, kind="ExternalInput")
    wq_d = nc.dram_tensor("wq", [SLOTS, 128, NT * 128], bf, kind="ExternalInput")
    wk_d = nc.dram_tensor("wk", [128, NT * 128], bf, kind="ExternalInput")
    wv_d = nc.dram_tensor("wv", [128, NT * 128], bf, kind="ExternalInput")
    kT_d = nc.dram_tensor("kT", [128, S_MAX], bf, kind="ExternalInput")
    v_d = nc.dram_tensor("v", [128, ST * 128], bf, kind="ExternalInput")
    trig_d = nc.dram_tensor("trig", [128, 4 * L], bf, kind="ExternalInput")
    bias_d = nc.dram_tensor("biases", [128, 6], f32, kind="ExternalInput")
    idrot_d = nc.dram_tensor("idrot", [128, 2 * 128], bf, kind="ExternalInput")
    wo_d = nc.dram_tensor("wo", [128, H * OSH], bf, kind="ExternalInput")
    out_d = nc.dram_tensor("out", [OSH, L], f32, kind="ExternalOutput")

    wt0 = cp // 128
    wset = set(range(wt0, wt0 + L // 128))
    # s-tile order: window (cache-update) tiles last
    SORD = [st for st in range(ST) if st not in wset] + sorted(wset)
    NCK = ST // 2                       # 16 chunks of 2 s-tiles per slot

    with tile.TileContext(nc) as tc, ExitStack() as ctx:
        const = ctx.enter_context(tc.tile_pool(name="const", bufs=1))
        persist = ctx.enter_context(tc.tile_pool(name="persist", bufs=1))
        kvpool = ctx.enter_context(tc.tile_pool(name="kvpool", bufs=1))
        wopool = ctx.enter_context(tc.tile_pool(name="wopool", bufs=1))
        agpool = ctx.enter_context(tc.tile_pool(name="agpool", bufs=1))
        spool = ctx.enter_context(tc.tile_pool(name="spool", bufs=2))
        ppool = ctx.enter_context(tc.tile_pool(name="ppool", bufs=4))
        accpool = ctx.enter_context(tc.tile_pool(name="accpool", bufs=2))
        pp = ctx.enter_context(tc.tile_pool(name="pp", bufs=1, space="PSUM"))
        dram = ctx.enter_context(tc.tile_pool(name="dram", bufs=1, space="DRAM"))

        ag_in = {g: dram.tile([nh * 128, L], bf, tag=f"agin{g}",
                              name=f"ag_in{g}")
                 for g, nh in (("A", 2), ("B", 1), ("C", 1))}
        ag_out = {g: dram.tile([NCORES * nh * 128, L], bf, tag=f"agout{g}",
                               name=f"ag_out{g}", addr_space="Shared")
                  for g, nh in (("A", 2), ("B", 1), ("C", 1))}

        # persistent SBUF
        K_T = kvpool.tile([128, S_MAX], bf, tag="kt", name="K_T")      # [d, s]
        v_sb = kvpool.tile([128, ST, 128], bf, tag="v", name="v_sb")   # [s,st,d]
        q_sb = [persist.tile([128, L], bf, tag=f"q{s}", name=f"q_sb{s}")
                for s in range(SLOTS)]
        osum = persist.tile([OSH // 4, 4, L], f32, tag="osum", name="osum")

        # ---- DMAs in priority order ----
        xw = ExitStack()
        xpool = xw.enter_context(tc.tile_pool(name="xpool", bufs=1))

        # first wave (one DMA per queue): x in 4 chunks + slot-0 weights +
        # first half of the K/V cache + trig
        x_sb = xpool.tile([128, NT, L], bf, tag="x", name="x_sb")
        x_r = x_d.rearrange("p (t l) -> p t l", l=L)
        for a, b in ((0, 7), (7, 14), (14, 21), (21, 28)):
            nc.sync.dma_start(out=x_sb[:, a:b], in_=x_r[:, a:b])
        wq_sb = []
        for s in range(SLOTS):
            w = xpool.tile([128, NT, 128], bf, tag=f"wq{s}", name=f"wq_sb{s}")
            wq_sb.append(w)
        nc.sync.dma_start(out=wq_sb[0][:],
                          in_=wq_d[0].rearrange("p (t d) -> p t d", d=128))
        trig = const.tile([128, 4, L], bf, tag="trig", name="trig")
        nc.sync.dma_start(out=trig[:], in_=trig_d.rearrange("p (i l) -> p i l", l=L))
        # K^T cache (non-window columns; host pre-transposed), V cache low half
        v_r = v_d.rearrange("p (t d) -> p t d", d=128)
        nc.sync.dma_start(out=K_T[:, 0:cp], in_=kT_d[:, 0:cp])
        nc.sync.dma_start(out=v_sb[:, 0:wt0], in_=v_r[:, 0:wt0])
        # second wave
        bia = const.tile([128, 6], f32, tag="bia", name="bia")
        nc.sync.dma_start(out=bia[:], in_=bias_d[:])
        idrot = const.tile([128, 2, 128], bf, tag="idrot", name="idrot")
        nc.sync.dma_start(out=idrot[:], in_=idrot_d.rearrange("p (i d) -> p i d", d=128))
        wk_sb = xpool.tile([128, NT, 128], bf, tag="wk", name="wk_sb")
        nc.sync.dma_start(out=wk_sb[:], in_=wk_d.rearrange("p (t d) -> p t d", d=128))
        wv_sb = xpool.tile([128, NT, 128], bf, tag="wv", name="wv_sb")
        nc.sync.dma_start(out=wv_sb[:], in_=wv_d.rearrange("p (t d) -> p t d", d=128))
        nc.sync.dma_start(out=K_T[:, cp + L:], in_=kT_d[:, cp + L:])
        nc.sync.dma_start(out=v_sb[:, wt0 + 4:], in_=v_r[:, wt0 + 4:])
        for s in range(1, SLOTS):
            nc.sync.dma_start(out=wq_sb[s][:],
                              in_=wq_d[s].rearrange("p (t d) -> p t d", d=128))
        woT_sb = wopool.tile([128, H, OSH], bf, name="woT_sb")
        nc.sync.dma_start(out=woT_sb[:], in_=wo_d.rearrange("p (g o) -> p g o", o=OSH))

        ones_bf = const.tile([128, 1], bf, tag="ones_bf", name="ones_bf")
        nc.gpsimd.memset(ones_bf[:], 1.0)
        onesr_bf = const.tile([1, 128], bf, tag="onesr_bf", name="onesr_bf")
        nc.gpsimd.memset(onesr_bf[:], 1.0)

        qcos, qsin = trig[:, 0, :], trig[:, 1, :]
        kcos, ksin = trig[:, 2, :], trig[:, 3, :]
        ident, rotm = idrot[:, 0, :], idrot[:, 1, :]

        # ---- helpers ----
        def proj(w_sb, name):
            ps = pp.tile([128, L], f32, tag="op2", bufs=2, name=f"ps_{name}")
            for t in range(NT):
                nc.tensor.matmul(ps[:], lhsT=w_sb[:, t, :], rhs=x_sb[:, t, :],
                                 start=(t == 0), stop=(t == NT - 1))
            return ps

        def rope(dst, ps, bcol, cos_t, sin_t, name):
            raw = spool.tile([128, L], bf, tag="raw", name=f"raw_{name}")
            nc.vector.tensor_scalar_add(raw[:], ps[:], bia[:, bcol:bcol + 1])
            rot_ps = pp.tile([128, L], f32, tag="sc", bufs=2, name=f"rot_{name}")
            nc.tensor.matmul(rot_ps[:], lhsT=rotm, rhs=raw[:], start=True,
                             stop=True)
            t1 = spool.tile([128, L], bf, tag="rt1", name=f"rt1_{name}")
            nc.vector.tensor_mul(t1[:], raw[:], cos_t)
            t2 = spool.tile([128, L], bf, tag="rt2", name=f"rt2_{name}")
            nc.vector.tensor_mul(t2[:], rot_ps[:], sin_t)
            nc.vector.tensor_add(dst, t1[:], t2[:])

        # ---- projections for slot 0, then k/v queued as attention filler ----
        q_ps0 = proj(wq_sb[0], "q0")
        rope(q_sb[0][:], q_ps0, 0, qcos, qsin, "q0")

        def kv_fill():
            # generator: yields after small batches so attention interleaves
            ps_k = pp.tile([128, L], f32, tag="op2", bufs=2, name="ps_k")
            for t in range(NT):
                nc.tensor.matmul(ps_k[:], lhsT=wk_sb[:, t, :], rhs=x_sb[:, t, :],
                                 start=(t == 0), stop=(t == NT - 1))
                if t % 7 == 6:
                    yield
            rope(K_T[:, cp:cp + L], ps_k, 4, kcos, ksin, "k")
            yield
            ps_v = pp.tile([128, L], f32, tag="op2", bufs=2, name="ps_v")
            for t in range(NT):
                nc.tensor.matmul(ps_v[:], lhsT=wv_sb[:, t, :], rhs=x_sb[:, t, :],
                                 start=(t == 0), stop=(t == NT - 1))
                if t % 7 == 6:
                    yield
            v_raw = spool.tile([128, L], bf, tag="vraw", name="v_raw")
            nc.vector.tensor_scalar_add(v_raw[:], ps_v[:], bia[:, 5:6])
            for lt in range(4):
                tp = pp.tile([128, 128], bf, tag="sc", bufs=2, name=f"tpv{lt}")
                nc.tensor.transpose(tp[:], v_raw[:, lt * 128:(lt + 1) * 128],
                                    ident)
                nc.vector.tensor_copy(v_sb[:, wt0 + lt, :], tp[:])
            yield
            # slot 1..3 q projections, in small batches
            for s in range(1, SLOTS):
                ps_q = pp.tile([128, L], f32, tag="op2", bufs=2, name=f"ps_q{s}")
                for t in range(NT):
                    nc.tensor.matmul(ps_q[:], lhsT=wq_sb[s][:, t, :],
                                     rhs=x_sb[:, t, :],
                                     start=(t == 0), stop=(t == NT - 1))
                    if t % 7 == 6:
                        yield
                rope(q_sb[s][:], ps_q, s, qcos, qsin, f"q{s}")
                yield

        filler = [kv_fill()]

        def run_filler(n=1):
            for _ in range(n):
                if not filler:
                    return
                try:
                    next(filler[0])
                except StopIteration:
                    filler.pop(0)

        # ---- o_proj machinery (filled in during slot-3 attention) ----
        attg = {}

        def oproj_rounds(group, first, last):
            ents = [(gi, e) for gi, e in enumerate(ENTRIES) if e[0] == group]
            for ot in range(4):
                m0 = ot * (OSH // 4)
                bank = pp.tile([OSH // 4, L], f32, tag="op2", bufs=2,
                               name=f"ob_{group}{ot}")
                for i, (gi, e) in enumerate(ents):
                    g, c, h = e
                    if g == "A":
                        rhs = attg["A"][:, c, h, :]
                    else:
                        rhs = attg[g][:, c, :]
                    nc.tensor.matmul(bank[:],
                                     lhsT=woT_sb[:, gi, m0:m0 + OSH // 4],
                                     rhs=rhs,
                                     start=(i == 0), stop=(i == len(ents) - 1))
                    if i % 6 == 5:
                        yield
                if first:
                    nc.vector.tensor_copy(osum[:, ot, :], bank[:])
                else:
                    nc.vector.tensor_add(osum[:, ot, :], osum[:, ot, :],
                                         bank[:])
                yield
            if last:
                for ot in range(4):
                    m0 = ot * (OSH // 4)
                    nc.sync.dma_start(out=out_d[m0:m0 + OSH // 4, :],
                                      in_=osum[:, ot, :])

        def load_attg(g, nh):
            agv = ag_out[g].rearrange("(c h p) l -> p c h l", c=NCORES, h=nh,
                                      p=128)
            ag_t = agpool.tile([128, NCORES, nh, L], bf, tag=f"attg{g}",
                               name=f"attg{g}")
            hc = NCORES // 2
            nc.sync.dma_start(out=ag_t[:, 0:hc], in_=agv[:, 0:hc])
            nc.sync.dma_start(out=ag_t[:, hc:], in_=agv[:, hc:])
            if nh == 1:
                attg[g] = ag_t.rearrange("p c h l -> p (c h) l")
            else:
                attg[g] = ag_t

        # ---- attention: one slot at a time ----
        def tail(s, acc, out_ps, g, row, psum_tag):
            den_ps = pp.tile([1, L], f32, tag=psum_tag, bufs=2, name=f"den{s}")
            nc.tensor.matmul(den_ps[:], lhsT=ones_bf[:], rhs=acc[:, 0:L],
                             start=True, stop=False)
            nc.tensor.matmul(den_ps[:], lhsT=ones_bf[:], rhs=acc[:, L:],
                             start=False, stop=True)
            den_sb = spool.tile([1, L], f32, tag="den_sb", name=f"den_sb{s}")
            nc.vector.tensor_copy(den_sb[:], den_ps[:])
            rec = spool.tile([1, L], f32, tag="rec", name=f"rec{s}")
            scr = spool.tile([1, L], f32, tag="scr", name=f"scr{s}")
            nc.vector.reciprocal_approx_accurate(rec[:], den_sb[:], scr[:])
            rec_bf = spool.tile([1, L], bf, tag="rec_bf", name=f"rec_bf{s}")
            nc.vector.tensor_copy(rec_bf[:], rec[:])
            bc_ps = pp.tile([128, L], f32, tag=psum_tag, bufs=2,
                            name=f"bc{s}")
            nc.tensor.matmul(bc_ps[:], lhsT=onesr_bf[:], rhs=rec_bf[:],
                             start=True, stop=True)
            bc_sb = spool.tile([128, L], f32, tag="bc_sb", name=f"bc_sb{s}")
            nc.vector.tensor_copy(bc_sb[:], bc_ps[:])
            att = spool.tile([128, L], bf, tag="att", bufs=2, name=f"att{s}")
            nc.vector.tensor_mul(att[:], out_ps[:], bc_sb[:])
            nc.sync.dma_start(out=ag_in[g][row * 128:(row + 1) * 128, :],
                              in_=att[:])

        def gather(g, nh):
            nc.gpsimd.collective_compute(
                "AllGather",
                mybir.AluOpType.bypass,
                replica_groups=[list(range(NCORES))],
                ins=[ag_in[g].opt()],
                outs=[ag_out[g].opt()],
            )

        for s in range(SLOTS):
            acc = accpool.tile([128, 2 * L], bf, tag="acc", name=f"acc{s}")
            out_ps = pp.tile([128, L], f32, tag="oab", bufs=2, name=f"out{s}")
            prev = None
            for ck in range(NCK):
                sa, sb = SORD[2 * ck], SORD[2 * ck + 1]
                sc = pp.tile([128, 2 * L], f32, tag="sc", bufs=2,
                             name=f"sc{s}_{ck}")
                nc.tensor.matmul(sc[:, 0:L],
                                 lhsT=K_T[:, sa * 128:(sa + 1) * 128],
                                 rhs=q_sb[s][:], start=True, stop=True)
                nc.tensor.matmul(sc[:, L:],
                                 lhsT=K_T[:, sb * 128:(sb + 1) * 128],
                                 rhs=q_sb[s][:], start=True, stop=True)
                # software pipeline: V-matmuls and den-accumulate of the
                # PREVIOUS chunk are issued after this chunk's score matmuls,
                # so the in-order PE queue never blocks on the current exp.
                if prev is not None:
                    pp_, pa_, pb_, pk_ = prev
                    nc.tensor.matmul(out_ps[:], lhsT=v_sb[:, pa_, :],
                                     rhs=pp_[:, 0:L],
                                     start=(pk_ == 0), stop=False)
                    nc.tensor.matmul(out_ps[:], lhsT=v_sb[:, pb_, :],
                                     rhs=pp_[:, L:],
                                     start=False, stop=False)
                p = ppool.tile([128, 2 * L], bf, tag="p", name=f"p{s}_{ck}")
                nc.scalar.activation(p[:], sc[:], AF.Exp, scale=SCALE)
                if prev is not None:
                    if prev[3] == 0:
                        nc.vector.tensor_copy(acc[:], prev[0][:])
                    else:
                        nc.vector.tensor_add(acc[:], acc[:], prev[0][:])
                run_filler(1)
                prev = (p, sa, sb, ck)
            pp_, pa_, pb_, pk_ = prev
            nc.tensor.matmul(out_ps[:], lhsT=v_sb[:, pa_, :], rhs=pp_[:, 0:L],
                             start=False, stop=False)
            nc.tensor.matmul(out_ps[:], lhsT=v_sb[:, pb_, :], rhs=pp_[:, L:],
                             start=False, stop=True)
            nc.vector.tensor_add(acc[:], acc[:], pp_[:])
            # group tails / gathers / o_proj interleave
            if s == 0:
                tail(s, acc, out_ps, "A", 0, "op2")
            elif s == 1:
                tail(s, acc, out_ps, "A", 1, "op2")
                gather("A", 2)
                load_attg("A", 2)
            elif s == 2:
                tail(s, acc, out_ps, "B", 0, "op2")
                gather("B", 1)
                load_attg("B", 1)
                filler.append(oproj_rounds("A", first=True, last=False))
            else:
                tail(s, acc, out_ps, "C", 0, "sc")
                gather("C", 1)
                load_attg("C", 1)

        # drain leftover filler, then remaining o_proj rounds
        while filler:
            run_filler(1)
        for _ in oproj_rounds("B", first=False, last=False):
            pass
        for _ in oproj_rounds("C", first=False, last=True):
            pass

        # exit x/wq scope (frees SBUF)
        xw.close()

    nc.compile()
    return nc


def _get_prog(cp):
    if cp not in _prog_cache:
        _prog_cache[cp] = _build(cp)
    return _prog_cache[cp]


def _shards(hidden_states, cos, sin, cos_t, sin_t, key_cache, value_cache,
            wq, bq, wk, bk, wv, bv, wo):
    import ml_dtypes
    f = np.float32
    b16 = ml_dtypes.bfloat16

    def tilemajor(wT):
        # [D, 128] (contraction-major) -> [128, NT*128] SBUF layout
        return np.ascontiguousarray(
            wT.reshape(NT, 128, -1).transpose(1, 0, 2).reshape(128, -1))

    x = hidden_states.reshape(D, L)
    x_arr = np.ascontiguousarray(
        x.reshape(NT, 128, L).transpose(1, 0, 2).reshape(128, NT * L)).astype(b16)
    qcos = np.asarray(cos_t, dtype=f).reshape(HD, L)
    qsin = np.asarray(sin_t, dtype=f).reshape(HD, L)
    kcos = np.asarray(cos, dtype=f).reshape(L, HD).T
    ksin = np.asarray(sin, dtype=f).reshape(L, HD).T
    trig = np.ascontiguousarray(
        np.concatenate([qcos, qsin, kcos, ksin], axis=1)).astype(b16)
    rotm = np.zeros((HD, HD), dtype=f)   # rot(q) = R @ q; pass R.T as lhsT
    half = HD // 2
    rotm[np.arange(half), np.arange(half) + half] = -1.0
    rotm[np.arange(half) + half, np.arange(half)] = 1.0
    idrot = np.ascontiguousarray(
        np.concatenate([np.eye(HD, dtype=f), rotm.T], axis=1)).astype(b16)

    maps = []
    for c in range(NCORES):
        kvh = c // 2
        wq_arr = np.zeros((SLOTS, 128, NT * 128), dtype=b16)
        biases = np.zeros((128, 6), dtype=f)
        for s in range(SLOTS):
            h = _head_of(c, s)
            if h is None:
                continue
            wq_arr[s] = tilemajor(
                np.ascontiguousarray(wq[h * HD:(h + 1) * HD, :].T)).astype(b16)
            biases[:, s] = bq[h * HD:(h + 1) * HD]
        biases[:, 4] = bk[kvh * HD:(kvh + 1) * HD]
        biases[:, 5] = bv[kvh * HD:(kvh + 1) * HD]
        kT = np.ascontiguousarray(key_cache[LI, kvh].T).astype(b16)
        vc = value_cache[LI, kvh]
        v_arr = np.ascontiguousarray(
            vc.reshape(ST, 128, HD).transpose(1, 0, 2).reshape(128, ST * HD)
        ).astype(b16)
        rows = slice(OSH * c, OSH * (c + 1))
        wo_arr = np.empty((128, H * OSH), dtype=b16)
        for gi, (g, cc, ss) in enumerate(ENTRIES):
            h = _head_of(cc, ss)
            wo_arr[:, gi * OSH:(gi + 1) * OSH] = \
                wo[rows, h * HD:(h + 1) * HD].T.astype(b16)
        maps.append({
            "x": x_arr,
            "wq": wq_arr,
            "wk": tilemajor(np.ascontiguousarray(
                wk[kvh * HD:(kvh + 1) * HD, :].T)).astype(b16),
            "wv": tilemajor(np.ascontiguousarray(
                wv[kvh * HD:(kvh + 1) * HD, :].T)).astype(b16),
            "kT": kT,
            "v": v_arr,
            "trig": trig,
            "biases": biases,
            "idrot": idrot,
            "wo": wo_arr,
        })
    return maps


def kernel(_trace=False, **inputs):
    from concourse.bass_utils import run_bass_kernel_spmd

    cp = int(np.asarray(inputs["cache_position"]))
    assert cp % 128 == 0 and 0 <= cp <= S_MAX - L, f"unsupported cache_position {cp}"

    maps = _shards(
        inputs["hidden_states"], inputs["cos"], inputs["sin"],
        inputs["cos_t"], inputs["sin_t"],
        inputs["key_cache"], inputs["value_cache"],
        inputs["wq"], inputs["bq"], inputs["wk"], inputs["bk"],
        inputs["wv"], inputs["bv"], inputs["wo"],
    )
    nc = _get_prog(cp)
    res = run_bass_kernel_spmd(nc, maps, core_ids=list(range(NCORES)),
                               trace=_trace)
    out = np.concatenate([r["out"] for r in res.results], axis=0)
    out = out.astype(np.float32).reshape(1, D, 1, L)
    if _trace:
        return out, res
    return out


# revision 17
# speedup vs baseline: 1.2572x; 1.2572x over previous
"""Bass/Tile TRN2 kernel for nn_AttentionANEWraperChannelsFirstWithCache.

Tensor-parallel over heads across 8 NeuronCores (v2):
  - 28 q heads in 4 slots/core; core c owns kv head c//2 (replicated per pair).
  - Head-slot groups processed sequentially (slot 0,1,2,3), each over the full
    4096-row cache with the cache-update window tiles ordered last so
    attention starts before the k/v projections finish.
  - exp chunks of [128, 1024] (2 s-tiles) on the scalar engine, double
    buffered in PSUM; softmax denominator accumulated on DVE in bf16;
    per-slot biases applied on DVE (tensor_scalar_add).
  - AllGather per slot-pair {0,1} / slot {2} / slot {3}; the first two overlap
    later attention; o_proj accumulates in 2 rotating PSUM banks with DVE
    flushes into an SBUF accumulator, interleaved into slot-3 attention.
  - K cache pre-transposed on host ([d, s]); x/wq/wk/wv/v-cache/woT laid out
    host-side so every DMA is contiguous per partition.

Matmul operands bf16 (fp32 PSUM), softmax stats fp32/bf16 mix.
"""

import math
import numpy as np

H, KV, HD, LI = 28, 4, 128, 5
S_MAX, D, L = 4096, 3584, 512
NCORES = 8
SLOTS = 4
OSH = D // NCORES          # 448 o_proj output rows per core
NT = D // 128              # 28 contraction tiles over hidden dim
ST = S_MAX // 128          # 32 s-tiles over the cache
SCALE = 1.0 / math.sqrt(HD)


def _head_of(core, slot):
    off = 4 * (core % 2) + slot
    if off >= 7:
        return None                      # dummy slot (odd cores, slot 3)
    return (core // 2) * 7 + off


# o_proj entry order = gather-buffer order, slot-major (one gather per slot;
# slot 3 exists only on even cores).
ENTRIES = [(s, c) for s in range(SLOTS) for c in range(NCORES)
           if _head_of(c, s) is not None]
assert len(ENTRIES) == H

_prog_cache = {}


def _build(cp):
    import concourse.bass as bass
    import concourse.mybir as mybir
    import concourse.tile as tile
    from concourse import bacc
    from contextlib import ExitStack

    f32 = mybir.dt.float32
    bf = mybir.dt.bfloat16
    AF = mybir.ActivationFunctionType
    nc = bacc.Bacc("TRN2", target_bir_lowering=False, debug=False,
                   num_devices=NCORES)

    x_d = nc.dram_tensor("x", [128, NT * L], bf, kind="ExternalInput")
    wq_d = nc.dram_tensor("wq", [SLOTS, 128, NT * 128], bf, kind="ExternalInput")
    wk_d = nc.dram_tensor("wk", [128, NT * 128], bf, kind="ExternalInput")
    wv_d = nc.dram_tensor("wv", [128, NT * 128], bf, kind="ExternalInput")
    kT_d = nc.dram_tensor("kT", [128, S_MAX], bf, kind="ExternalInput")
    v_d = nc.dram_tensor("v", [128, ST * 128], bf, kind="ExternalInput")
    trig_d = nc.dram_tensor("trig", [128, 4 * L], bf, kind="ExternalInput")
    bias_d = nc.dram_tensor("biases", [128, 6], f32, kind="ExternalInput")
    idrot_d = nc.dram_tensor("idrot", [128, 2 * 128], bf, kind="ExternalInput")
    wo_d = nc.dram_tensor("wo", [128, H * OSH], bf, kind="ExternalInput")
    out_d = nc.dram_tensor("out", [OSH, L], f32, kind="ExternalOutput")

    wt0 = cp // 128
    wset = set(range(wt0, wt0 + L // 128))
    # s-tile order: window (cache-update) tiles last
    SORD = [st for st in range(ST) if st not in wset] + sorted(wset)
    NCK = ST // 2                       # 16 chunks of 2 s-tiles per slot

    with tile.TileContext(nc) as tc, ExitStack() as ctx:
        const = ctx.enter_context(tc.tile_pool(name="const", bufs=1))
        persist = ctx.enter_context(tc.tile_pool(name="persist", bufs=1))
        kvpool = ctx.enter_context(tc.tile_pool(name="kvpool", bufs=1))
        wopool = ctx.enter_context(tc.tile_pool(name="wopool", bufs=1))
        agpool = ctx.enter_context(tc.tile_pool(name="agpool", bufs=1))
        spool = ctx.enter_context(tc.tile_pool(name="spool", bufs=2))
        ppool = ctx.enter_context(tc.tile_pool(name="ppool", bufs=4))
        accpool = ctx.enter_context(tc.tile_pool(name="accpool", bufs=2))
        pp = ctx.enter_context(tc.tile_pool(name="pp", bufs=1, space="PSUM"))
        dram = ctx.enter_context(tc.tile_pool(name="dram", bufs=1, space="DRAM"))

        ag_in = {s: dram.tile([128, L], bf, tag=f"agin{s}", name=f"ag_in{s}")
                 for s in range(SLOTS)}
        ag_out = {s: dram.tile([NCORES * 128, L], bf, tag=f"agout{s}",
                               name=f"ag_out{s}", addr_space="Shared")
                  for s in range(SLOTS)}

        # persistent SBUF
        K_T = kvpool.tile([128, S_MAX], bf, tag="kt", name="K_T")      # [d, s]
        v_sb = kvpool.tile([128, ST, 128], bf, tag="v", name="v_sb")   # [s,st,d]
        q_sb = [persist.tile([128, L], bf, tag=f"q{s}", name=f"q_sb{s}")
                for s in range(SLOTS)]
        osum = persist.tile([OSH // 4, 4, L], f32, tag="osum", name="osum")

        # ---- DMAs in priority order ----
        xw = ExitStack()
        xpool = xw.enter_context(tc.tile_pool(name="xpool", bufs=1))

        # first wave (one DMA per queue): x in 4 chunks + slot-0 weights +
        # first half of the K/V cache + trig
        x_sb = xpool.tile([128, NT, L], bf, tag="x", name="x_sb")
        x_r = x_d.rearrange("p (t l) -> p t l", l=L)
        for a, b in ((0, 7), (7, 14), (14, 21), (21, 28)):
            nc.sync.dma_start(out=x_sb[:, a:b], in_=x_r[:, a:b])
        wq_sb = []
        for s in range(SLOTS):
            w = xpool.tile([128, NT, 128], bf, tag=f"wq{s}", name=f"wq_sb{s}")
            wq_sb.append(w)
        nc.sync.dma_start(out=wq_sb[0][:],
                          in_=wq_d[0].rearrange("p (t d) -> p t d", d=128))
        trig = const.tile([128, 4, L], bf, tag="trig", name="trig")
        nc.sync.dma_start(out=trig[:], in_=trig_d.rearrange("p (i l) -> p i l", l=L))
        # K^T cache (non-window columns; host pre-transposed), V cache low half
        v_r = v_d.rearrange("p (t d) -> p t d", d=128)
        nc.sync.dma_start(out=K_T[:, 0:cp], in_=kT_d[:, 0:cp])
        nc.sync.dma_start(out=v_sb[:, 0:wt0], in_=v_r[:, 0:wt0])
        # second wave
        bia = const.tile([128, 6], f32, tag="bia", name="bia")
        nc.sync.dma_start(out=bia[:], in_=bias_d[:])
        idrot = const.tile([128, 2, 128], bf, tag="idrot", name="idrot")
        nc.sync.dma_start(out=idrot[:], in_=idrot_d.rearrange("p (i d) -> p i d", d=128))
        wk_sb = xpool.tile([128, NT, 128], bf, tag="wk", name="wk_sb")
        nc.sync.dma_start(out=wk_sb[:], in_=wk_d.rearrange("p (t d) -> p t d", d=128))
        wv_sb = xpool.tile([128, NT, 128], bf, tag="wv", name="wv_sb")
        nc.sync.dma_start(out=wv_sb[:], in_=wv_d.rearrange("p (t d) -> p t d", d=128))
        nc.sync.dma_start(out=K_T[:, cp + L:], in_=kT_d[:, cp + L:])
        nc.sync.dma_start(out=v_sb[:, wt0 + 4:], in_=v_r[:, wt0 + 4:])
        for s in range(1, SLOTS):
            nc.sync.dma_start(out=wq_sb[s][:],
                              in_=wq_d[s].rearrange("p (t d) -> p t d", d=128))
        woT_sb = wopool.tile([128, H, OSH], bf, name="woT_sb")
        nc.sync.dma_start(out=woT_sb[:], in_=wo_d.rearrange("p (g o) -> p g o", o=OSH))

        ones_bf = const.tile([128, 1], bf, tag="ones_bf", name="ones_bf")
        nc.gpsimd.memset(ones_bf[:], 1.0)
        onesr_bf = const.tile([1, 128], bf, tag="onesr_bf", name="onesr_bf")
        nc.gpsimd.memset(onesr_bf[:], 1.0)

        qcos, qsin = trig[:, 0, :], trig[:, 1, :]
        kcos, ksin = trig[:, 2, :], trig[:, 3, :]
        ident, rotm = idrot[:, 0, :], idrot[:, 1, :]

        # ---- helpers ----
        def proj(w_sb, name):
            ps = pp.tile([128, L], f32, tag="op2", bufs=2, name=f"ps_{name}")
            for t in range(NT):
                nc.tensor.matmul(ps[:], lhsT=w_sb[:, t, :], rhs=x_sb[:, t, :],
                                 start=(t == 0), stop=(t == NT - 1))
            return ps

        def rope(dst, ps, bcol, cos_t, sin_t, name):
            raw = spool.tile([128, L], bf, tag="raw", name=f"raw_{name}")
            nc.vector.tensor_scalar_add(raw[:], ps[:], bia[:, bcol:bcol + 1])
            rot_ps = pp.tile([128, L], f32, tag="sc", bufs=2, name=f"rot_{name}")
            nc.tensor.matmul(rot_ps[:], lhsT=rotm, rhs=raw[:], start=True,
                             stop=True)
            t1 = spool.tile([128, L], bf, tag="rt1", name=f"rt1_{name}")
            nc.vector.tensor_mul(t1[:], raw[:], cos_t)
            t2 = spool.tile([128, L], bf, tag="rt2", name=f"rt2_{name}")
            nc.vector.tensor_mul(t2[:], rot_ps[:], sin_t)
            nc.vector.tensor_add(dst, t1[:], t2[:])

        # ---- projections for slot 0, then k/v queued as attention filler ----
        q_ps0 = proj(wq_sb[0], "q0")
        rope(q_sb[0][:], q_ps0, 0, qcos, qsin, "q0")

        def kv_fill():
            # generator: yields after small batches so attention interleaves
            ps_k = pp.tile([128, L], f32, tag="op2", bufs=2, name="ps_k")
            for t in range(NT):
                nc.tensor.matmul(ps_k[:], lhsT=wk_sb[:, t, :], rhs=x_sb[:, t, :],
                                 start=(t == 0), stop=(t == NT - 1))
                if t % 7 == 6:
                    yield
            rope(K_T[:, cp:cp + L], ps_k, 4, kcos, ksin, "k")
            yield
            ps_v = pp.tile([128, L], f32, tag="op2", bufs=2, name="ps_v")
            for t in range(NT):
                nc.tensor.matmul(ps_v[:], lhsT=wv_sb[:, t, :], rhs=x_sb[:, t, :],
                                 start=(t == 0), stop=(t == NT - 1))
                if t % 7 == 6:
                    yield
            v_raw = spool.tile([128, L], bf, tag="vraw", name="v_raw")
            nc.vector.tensor_scalar_add(v_raw[:], ps_v[:], bia[:, 5:6])
            for lt in range(4):
                tp = pp.tile([128, 128], bf, tag="sc", bufs=2, name=f"tpv{lt}")
                nc.tensor.transpose(tp[:], v_raw[:, lt * 128:(lt + 1) * 128],
                                    ident)
                nc.vector.tensor_copy(v_sb[:, wt0 + lt, :], tp[:])
            yield
            # slot 1..3 q projections, in small batches
            for s in range(1, SLOTS):
                ps_q = pp.tile([128, L], f32, tag="op2", bufs=2, name=f"ps_q{s}")
                for t in range(NT):
                    nc.tensor.matmul(ps_q[:], lhsT=wq_sb[s][:, t, :],
                                     rhs=x_sb[:, t, :],
                                     start=(t == 0), stop=(t == NT - 1))
                    if t % 7 == 6:
                        yield
                rope(q_sb[s][:], ps_q, s, qcos, qsin, f"q{s}")
                yield

        filler = [kv_fill()]

        def run_filler(n=1):
            for _ in range(n):
                if not filler:
                    return
                try:
                    next(filler[0])
                except StopIteration:
                    filler.pop(0)

        # ---- o_proj machinery (filled in during slot-3 attention) ----
        attg = {}

        def oproj_rounds(group, first, last):
            ents = [(gi, e) for gi, e in enumerate(ENTRIES) if e[0] == group]
            for ot in range(4):
                m0 = ot * (OSH // 4)
                bank = pp.tile([OSH // 4, L], f32, tag="op2", bufs=2,
                               name=f"ob_{group}{ot}")
                for i, (gi, e) in enumerate(ents):
                    g, c = e
                    nc.tensor.matmul(bank[:],
                                     lhsT=woT_sb[:, gi, m0:m0 + OSH // 4],
                                     rhs=attg[g][:, c, :],
                                     start=(i == 0), stop=(i == len(ents) - 1))
                    if i % 4 == 3:
                        yield
                if first:
                    nc.vector.tensor_copy(osum[:, ot, :], bank[:])
                else:
                    nc.vector.tensor_add(osum[:, ot, :], osum[:, ot, :],
                                         bank[:])
                yield
            if last:
                for ot in range(4):
                    m0 = ot * (OSH // 4)
                    nc.sync.dma_start(out=out_d[m0:m0 + OSH // 4, :],
                                      in_=osum[:, ot, :])

        def load_attg(g):
            agv = ag_out[g].rearrange("(c p) l -> p c l", c=NCORES, p=128)
            ag_t = agpool.tile([128, NCORES, L], bf, tag=f"attg{g}",
                               name=f"attg{g}")
            hc = NCORES // 2
            nc.sync.dma_start(out=ag_t[:, 0:hc], in_=agv[:, 0:hc])
            nc.sync.dma_start(out=ag_t[:, hc:], in_=agv[:, hc:])
            attg[g] = ag_t

        # ---- attention: one slot at a time ----
        def tail(s, acc, out_ps, psum_tag):
            den_ps = pp.tile([1, L], f32, tag=psum_tag, bufs=2, name=f"den{s}")
            nc.tensor.matmul(den_ps[:], lhsT=ones_bf[:], rhs=acc[:, 0:L],
                             start=True, stop=False)
            nc.tensor.matmul(den_ps[:], lhsT=ones_bf[:], rhs=acc[:, L:],
                             start=False, stop=True)
            den_sb = spool.tile([1, L], f32, tag="den_sb", name=f"den_sb{s}")
            nc.vector.tensor_copy(den_sb[:], den_ps[:])
            rec = spool.tile([1, L], f32, tag="rec", name=f"rec{s}")
            scr = spool.tile([1, L], f32, tag="scr", name=f"scr{s}")
            nc.vector.reciprocal_approx_accurate(rec[:], den_sb[:], scr[:])
            rec_bf = spool.tile([1, L], bf, tag="rec_bf", name=f"rec_bf{s}")
            nc.vector.tensor_copy(rec_bf[:], rec[:])
            bc_ps = pp.tile([128, L], f32, tag=psum_tag, bufs=2,
                            name=f"bc{s}")
            nc.tensor.matmul(bc_ps[:], lhsT=onesr_bf[:], rhs=rec_bf[:],
                             start=True, stop=True)
            bc_sb = spool.tile([128, L], f32, tag="bc_sb", name=f"bc_sb{s}")
            nc.vector.tensor_copy(bc_sb[:], bc_ps[:])
            att = spool.tile([128, L], bf, tag="att", bufs=2, name=f"att{s}")
            nc.vector.tensor_mul(att[:], out_ps[:], bc_sb[:])
            nc.sync.dma_start(out=ag_in[s][:], in_=att[:])

        def gather(g):
            nc.gpsimd.collective_compute(
                "AllGather",
                mybir.AluOpType.bypass,
                replica_groups=[list(range(NCORES))],
                ins=[ag_in[g].opt()],
                outs=[ag_out[g].opt()],
            )

        # o_proj rounds for slot s are consumed as PE filler two slots later
        # (chunk 3 onward), giving each gather ~40us of slack before any PE
        # instruction depends on it.
        oproj_fill = {2: [], 3: []}
        for s in range(SLOTS):
            acc = accpool.tile([128, 2 * L], bf, tag="acc", name=f"acc{s}")
            out_ps = pp.tile([128, L], f32, tag="oab", bufs=2, name=f"out{s}")
            if s in oproj_fill:
                fq = oproj_fill[s]
            else:
                fq = None
            prev = None
            for ck in range(NCK):
                sa, sb = SORD[2 * ck], SORD[2 * ck + 1]
                sc = pp.tile([128, 2 * L], f32, tag="sc", bufs=2,
                             name=f"sc{s}_{ck}")
                nc.tensor.matmul(sc[:, 0:L],
                                 lhsT=K_T[:, sa * 128:(sa + 1) * 128],
                                 rhs=q_sb[s][:], start=True, stop=True)
                nc.tensor.matmul(sc[:, L:],
                                 lhsT=K_T[:, sb * 128:(sb + 1) * 128],
                                 rhs=q_sb[s][:], start=True, stop=True)
                # software pipeline: V-matmuls and den-accumulate of the
                # PREVIOUS chunk are issued after this chunk's score matmuls,
                # so the in-order PE queue never blocks on the current exp.
                if prev is not None:
                    pp_, pa_, pb_, pk_ = prev
                    nc.tensor.matmul(out_ps[:], lhsT=v_sb[:, pa_, :],
                                     rhs=pp_[:, 0:L],
                                     start=(pk_ == 0), stop=False)
                    nc.tensor.matmul(out_ps[:], lhsT=v_sb[:, pb_, :],
                                     rhs=pp_[:, L:],
                                     start=False, stop=False)
                p = ppool.tile([128, 2 * L], bf, tag="p", name=f"p{s}_{ck}")
                nc.scalar.activation(p[:], sc[:], AF.Exp, scale=SCALE)
                if prev is not None:
                    if prev[3] == 0:
                        nc.vector.tensor_copy(acc[:], prev[0][:])
                    else:
                        nc.vector.tensor_add(acc[:], acc[:], prev[0][:])
                run_filler(1)
                if fq and ck >= 3:
                    try:
                        next(fq[0])
                    except StopIteration:
                        fq.pop(0)
                prev = (p, sa, sb, ck)
            pp_, pa_, pb_, pk_ = prev
            nc.tensor.matmul(out_ps[:], lhsT=v_sb[:, pa_, :], rhs=pp_[:, 0:L],
                             start=False, stop=False)
            nc.tensor.matmul(out_ps[:], lhsT=v_sb[:, pb_, :], rhs=pp_[:, L:],
                             start=False, stop=True)
            nc.vector.tensor_add(acc[:], acc[:], pp_[:])
            # per-slot tail, gather, and deferred o_proj rounds
            tail(s, acc, out_ps, "sc" if s == SLOTS - 1 else "op2")
            gather(s)
            load_attg(s)
            if s + 2 in oproj_fill:
                oproj_fill[s + 2].append(
                    oproj_rounds(s, first=(s == 0), last=False))

        # drain leftover filler; slot-2 o_proj covers PE during gather 3
        while filler:
            run_filler(1)
        for q in (oproj_fill[2], oproj_fill[3]):
            for gen in q:
                for _ in gen:
                    pass
        for _ in oproj_rounds(2, first=False, last=False):
            pass
        for _ in oproj_rounds(3, first=False, last=True):
            pass

        # exit x/wq scope (frees SBUF)
        xw.close()

    nc.compile()
    return nc


def _get_prog(cp):
    if cp not in _prog_cache:
        _prog_cache[cp] = _build(cp)
    return _prog_cache[cp]


def _shards(hidden_states, cos, sin, cos_t, sin_t, key_cache, value_cache,
            wq, bq, wk, bk, wv, bv, wo):
    import ml_dtypes
    f = np.float32
    b16 = ml_dtypes.bfloat16

    def tilemajor(wT):
        # [D, 128] (contraction-major) -> [128, NT*128] SBUF layout
        return np.ascontiguousarray(
            wT.reshape(NT, 128, -1).transpose(1, 0, 2).reshape(128, -1))

    x = hidden_states.reshape(D, L)
    x_arr = np.ascontiguousarray(
        x.reshape(NT, 128, L).transpose(1, 0, 2).reshape(128, NT * L)).astype(b16)
    qcos = np.asarray(cos_t, dtype=f).reshape(HD, L)
    qsin = np.asarray(sin_t, dtype=f).reshape(HD, L)
    kcos = np.asarray(cos, dtype=f).reshape(L, HD).T
    ksin = np.asarray(sin, dtype=f).reshape(L, HD).T
    trig = np.ascontiguousarray(
        np.concatenate([qcos, qsin, kcos, ksin], axis=1)).astype(b16)
    rotm = np.zeros((HD, HD), dtype=f)   # rot(q) = R @ q; pass R.T as lhsT
    half = HD // 2
    rotm[np.arange(half), np.arange(half) + half] = -1.0
    rotm[np.arange(half) + half, np.arange(half)] = 1.0
    idrot = np.ascontiguousarray(
        np.concatenate([np.eye(HD, dtype=f), rotm.T], axis=1)).astype(b16)

    maps = []
    for c in range(NCORES):
        kvh = c // 2
        wq_arr = np.zeros((SLOTS, 128, NT * 128), dtype=b16)
        biases = np.zeros((128, 6), dtype=f)
        for s in range(SLOTS):
            h = _head_of(c, s)
            if h is None:
                continue
            wq_arr[s] = tilemajor(
                np.ascontiguousarray(wq[h * HD:(h + 1) * HD, :].T)).astype(b16)
            biases[:, s] = bq[h * HD:(h + 1) * HD]
        biases[:, 4] = bk[kvh * HD:(kvh + 1) * HD]
        biases[:, 5] = bv[kvh * HD:(kvh + 1) * HD]
        kT = np.ascontiguousarray(key_cache[LI, kvh].T).astype(b16)
        vc = value_cache[LI, kvh]
        v_arr = np.ascontiguousarray(
            vc.reshape(ST, 128, HD).transpose(1, 0, 2).reshape(128, ST * HD)
        ).astype(b16)
        rows = slice(OSH * c, OSH * (c + 1))
        wo_arr = np.empty((128, H * OSH), dtype=b16)
        for gi, (ss, cc) in enumerate(ENTRIES):
            h = _head_of(cc, ss)
            wo_arr[:, gi * OSH:(gi + 1) * OSH] = \
                wo[rows, h * HD:(h + 1) * HD].T.astype(b16)
        maps.append({
            "x": x_arr,
            "wq": wq_arr,
            "wk": tilemajor(np.ascontiguousarray(
                wk[kvh * HD:(kvh + 1) * HD, :].T)).astype(b16),
            "wv": tilemajor(np.ascontiguousarray(
                wv[kvh * HD:(kvh + 1) * HD, :].T)).astype(b16),
            "kT": kT,
            "v": v_arr,
            "trig": trig,
            "biases": biases,
            "idrot": idrot,
            "wo": wo_arr,
        })
    return maps


def kernel(_trace=False, **inputs):
    from concourse.bass_utils import run_bass_kernel_spmd

    cp = int(np.asarray(inputs["cache_position"]))
    assert cp % 128 == 0 and 0 <= cp <= S_MAX - L, f"unsupported cache_position {cp}"

    maps = _shards(
        inputs["hidden_states"], inputs["cos"], inputs["sin"],
        inputs["cos_t"], inputs["sin_t"],
        inputs["key_cache"], inputs["value_cache"],
        inputs["wq"], inputs["bq"], inputs["wk"], inputs["bk"],
        inputs["wv"], inputs["bv"], inputs["wo"],
    )
    nc = _get_prog(cp)
    res = run_bass_kernel_spmd(nc, maps, core_ids=list(range(NCORES)),
                               trace=_trace)
    out = np.concatenate([r["out"] for r in res.results], axis=0)
    out = out.astype(np.float32).reshape(1, D, 1, L)
    if _trace:
        return out, res
    return out
